# revision 18
# baseline (speedup 1.0000x reference)
"""Causal self-attention (B=2, T=2048, C=1024, H=16) on 8 trn2 NeuronCores.

Sharding: core = b*4 + g  ->  batch b, heads 4g..4g+3 (tensor-parallel on the
head/C dimension of the QKV and output projections).  Each core computes full-T
causal attention for its 4 heads and a partial output projection; the host sums
the 4 partials per batch and adds bo.

Device layout (per core):
  xt   [C, T]    x[b] transposed (host-side)
  QT/KT [256, T] head-major, d on partitions (2 head-"pairs" of 128 partitions)
  V    [T, 256]  natural layout (T on partitions, 16 tiles of [128, 256])
  S^T = K @ Q^T per head: [T_k, T_q] blocks of [128, 512]; both heads of a pair
       computed concurrently via row-tiled matmuls (K=64 each, tile_position
       (0,0)/(64,0)) into one [128, 1024] PSUM tile; one ACT exp per pair.
  Causal masking at 128x512 block granularity; diagonal blocks multiplied by
       one of 4 static masks (bf16, host-provided).
  Denominators: 4 col-tiled ones-matmuls (M=1 at partitions 0/32/64/96) per
       (q,k) block accumulating in one PSUM bank.
  O'^T accumulates over k in PSUM per pair ([128, 512]); normalized by
       1/denom broadcast (gpsimd partition_broadcast + DVE fast reciprocal).
  ypart[T, C] = O^T.T @ Wo_slice accumulated over the two 128-row chunks.

All matmuls run as float32r (fp32 bits, FP22-truncated multiply) -> 1 cycle/row
at N>=256, fp32 accumulation in PSUM.
"""

import numpy as np
import ml_dtypes

import concourse.bass as bass
import concourse.mybir as mybir
import concourse.tile as tile
from concourse import bacc
from concourse.bass_utils import run_bass_kernel_spmd
from concourse.dve_ops import RECIPROCAL_APPROX_FAST, RECIP_APPROX_FAST_CONSTS

B, T, C, H, D = 2, 2048, 1024, 16, 64
N_CORES = 8
HS = 256              # head-dim slice per core (4 heads x 64)
HSV = 260             # V slice width: 4 heads x (64 + ones column)
NQ = T // 512         # 4 q-tiles of 512
NK = T // 128         # 16 k-tiles of 128
NC8 = C // 128        # 8 contraction chunks
F32 = mybir.dt.float32
F32R = mybir.dt.float32r
BF16 = mybir.dt.bfloat16

_CACHE = {}


def _r(ap):
    return ap.bitcast(F32R)


def _build():
    nc = bacc.Bacc("TRN2", target_bir_lowering=False, debug=False,
                   num_devices=N_CORES)

    xt_d = nc.dram_tensor("xt", [C, T], F32R, kind="ExternalInput")
    wq_d = nc.dram_tensor("wq", [C, HS], F32R, kind="ExternalInput")
    wk_d = nc.dram_tensor("wk", [C, HS], F32R, kind="ExternalInput")
    wv_d = nc.dram_tensor("wv", [C, HSV], F32R, kind="ExternalInput")
    wo_d = nc.dram_tensor("wo", [HS, C], F32R, kind="ExternalInput")
    bq_d = nc.dram_tensor("bq", [128, 2], F32, kind="ExternalInput")
    bk_d = nc.dram_tensor("bk", [128, 2], F32, kind="ExternalInput")
    bv_d = nc.dram_tensor("bv", [128, HSV], F32, kind="ExternalInput")
    ms_d = nc.dram_tensor("ms", [4, 128, 1024], BF16, kind="ExternalInput")
    on_d = nc.dram_tensor("on", [128, 64], F32R, kind="ExternalInput")
    y_d = nc.dram_tensor("y", [T, C], F32, kind="ExternalOutput")

    with tile.TileContext(nc) as tc:
        with (
            tc.tile_pool(name="const", bufs=1) as cpool,
            tc.tile_pool(name="pp", bufs=3) as ppool,
            tc.tile_pool(name="onorm", bufs=4) as opool,
            tc.tile_pool(name="bc", bufs=2) as bcpool,
            tc.tile_pool(name="outp", bufs=4) as outpool,
            tc.tile_pool(name="spsum", bufs=1, space="PSUM") as spool,
            tc.tile_pool(name="opsum", bufs=1, space="PSUM") as oppool,
            tc.tile_pool(name="gpsum", bufs=2, space="PSUM") as gpool,
        ):
            # ---- persistent SBUF tensors ----
            xt_s = cpool.tile([128, NC8 * T], F32, tag="xt")
            wq_s = cpool.tile([128, NC8 * HS], F32, tag="wq")
            wk_s = cpool.tile([128, NC8 * HS], F32, tag="wk")
            wv_s = cpool.tile([128, NC8 * HSV], F32, tag="wv")
            wo_s = cpool.tile([128, 2 * C], F32, tag="wo")
            v_s = cpool.tile([128, NK * HSV], F32, tag="vs")
            qt_s = [cpool.tile([128, T], F32, tag=f"qt{p}", name=f"qt{p}")
                    for p in range(2)]
            kt_s = [cpool.tile([128, T], F32, tag=f"kt{p}", name=f"kt{p}")
                    for p in range(2)]
            ms_s = cpool.tile([128, 4 * 1024], BF16, tag="ms")
            bq_s = cpool.tile([128, 2], F32, tag="bq")
            bk_s = cpool.tile([128, 2], F32, tag="bk")
            bv_s = cpool.tile([128, HSV], F32, tag="bv")
            on_s = cpool.tile([128, 64], F32, tag="on")

            # ---- input DMAs ----
            for c in range(NC8):
                nc.sync.dma_start(out=_r(xt_s[:, T * c:T * (c + 1)]),
                                  in_=xt_d.ap()[128 * c:128 * (c + 1), :])
            for w_s, w_d, ww in ((wq_s, wq_d, HS), (wk_s, wk_d, HS),
                                 (wv_s, wv_d, HSV)):
                for c in range(NC8):
                    nc.sync.dma_start(out=_r(w_s[:, ww * c:ww * (c + 1)]),
                                      in_=w_d.ap()[128 * c:128 * (c + 1), :])
            for p in range(2):
                nc.sync.dma_start(out=_r(wo_s[:, C * p:C * (p + 1)]),
                                  in_=wo_d.ap()[128 * p:128 * (p + 1), :])
            for m in range(4):
                nc.sync.dma_start(out=ms_s[:, 1024 * m:1024 * (m + 1)],
                                  in_=ms_d.ap()[m])
            nc.sync.dma_start(out=bq_s[:], in_=bq_d.ap())
            nc.sync.dma_start(out=bk_s[:], in_=bk_d.ap())
            nc.sync.dma_start(out=bv_s[:], in_=bv_d.ap())
            nc.sync.dma_start(out=_r(on_s[:]), in_=on_d.ap())

            def emit_qkv_block(n):
                """QT/KT for q/k-range [512n, 512n+512) and V t-tiles 4n..4n+3."""
                for p in range(2):
                    for w_s, b_s, t_s in ((wq_s, bq_s, qt_s), (wk_s, bk_s, kt_s)):
                        ps = gpool.tile([128, 512], F32, tag="g")
                        for c in range(NC8):
                            nc.tensor.matmul(
                                ps[:],
                                _r(w_s[:, HS * c + 128 * p:HS * c + 128 * (p + 1)]),
                                _r(xt_s[:, T * c + 512 * n:T * c + 512 * (n + 1)]),
                                start=(c == 0), stop=(c == NC8 - 1))
                        nc.vector.tensor_scalar_add(
                            out=_r(t_s[p][:, 512 * n:512 * (n + 1)]),
                            in0=ps[:], scalar1=b_s[:, p:p + 1])
                for u in range(4):
                    t_idx = 4 * n + u
                    ps = gpool.tile([128, HSV], F32, tag="g")
                    for c in range(NC8):
                        nc.tensor.matmul(
                            ps[:],
                            _r(xt_s[:, T * c + 128 * t_idx:T * c + 128 * (t_idx + 1)]),
                            _r(wv_s[:, HSV * c:HSV * (c + 1)]),
                            start=(c == 0), stop=(c == NC8 - 1))
                    nc.vector.tensor_add(
                        out=_r(v_s[:, HSV * t_idx:HSV * (t_idx + 1)]),
                        in0=ps[:], in1=bv_s[:])

            def emit_attention_block(j):
                """Causal attention + output projection for q-tile j."""
                qsl = slice(512 * j, 512 * (j + 1))
                nk = 4 * (j + 1)
                o_ps = [oppool.tile([128, 512], F32, tag=f"o{h}", name=f"o_ps{h}_{j}")
                        for h in range(4)]
                for k in range(nk):
                    ksl = slice(128 * k, 128 * (k + 1))
                    p_t = []
                    for p in range(2):
                        s_ps = spool.tile([128, 1024], F32, tag="s",
                                          name=f"s_{j}_{k}_{p}")
                        for e in range(2):
                            nc.tensor.matmul(
                                s_ps[:, 512 * e:512 * (e + 1)],
                                _r(kt_s[p][64 * e:64 * (e + 1), ksl]),
                                _r(qt_s[p][64 * e:64 * (e + 1), qsl]),
                                start=True, stop=True,
                                tile_position=(64 * e, 0))
                        pt = ppool.tile([128, 1024], F32, tag="p",
                                        name=f"p_{j}_{k}_{p}")
                        nc.scalar.activation(_r(pt[:]), s_ps[:],
                                             mybir.ActivationFunctionType.Exp)
                        if k >= 4 * j:
                            m = k - 4 * j
                            nc.vector.tensor_mul(
                                _r(pt[:]), pt[:],
                                ms_s[:, 1024 * m:1024 * (m + 1)])
                        p_t.append(pt)
                    for h in range(4):
                        p, e = divmod(h, 2)
                        nc.tensor.matmul(
                            o_ps[h][0:65, :],
                            _r(v_s[:, HSV * k + 65 * h:HSV * k + 65 * (h + 1)]),
                            _r(p_t[p][:, 512 * e:512 * (e + 1)]),
                            start=(k == 0), stop=(k == nk - 1))
                # normalize:  O = O' * (1/denom),  then project
                dn = bcpool.tile([128, 512], F32, tag="dn", name=f"dn_{j}")
                for h in range(4):
                    nc.vector.tensor_copy(dn[32 * h:32 * h + 1, :],
                                          o_ps[h][64:65, :])
                rr = bcpool.tile([128, 512], F32, tag="rr", name=f"rr_{j}")
                nc.vector._custom_dve(
                    RECIPROCAL_APPROX_FAST, out=_r(rr[:]), in0=dn[:],
                    s0=RECIP_APPROX_FAST_CONSTS["s0"],
                    s1=RECIP_APPROX_FAST_CONSTS["s1"],
                    imm2=RECIP_APPROX_FAST_CONSTS["imm2"])
                onorm = []
                for p in range(2):
                    bc = bcpool.tile([128, 512], F32, tag="bc", name=f"bc_{j}_{p}")
                    ot = opool.tile([128, 512], F32, tag="onorm",
                                    name=f"onorm_{j}_{p}")
                    for e in range(2):
                        h = 2 * p + e
                        bc_ps = gpool.tile([128, 512], F32, tag="g",
                                           name=f"bcps_{j}_{h}")
                        nc.tensor.matmul(
                            bc_ps[0:64, :],
                            _r(on_s[32 * h:32 * h + 1, :]),
                            _r(rr[32 * h:32 * h + 1, :]),
                            start=True, stop=True,
                            tile_position=(32 * h, 0))
                        nc.vector.tensor_copy(bc[64 * e:64 * (e + 1), :],
                                              bc_ps[0:64, :])
                        nc.vector.tensor_copy(_r(ot[64 * e:64 * (e + 1), :]),
                                              o_ps[h][0:64, :])
                    nc.vector.tensor_mul(_r(ot[:]), ot[:], bc[:])
                    onorm.append(ot)
                for u in range(4):
                    for n2 in range(2):
                        y_ps = gpool.tile([128, 512], F32, tag="g")
                        for p in range(2):
                            nc.tensor.matmul(
                                y_ps[:],
                                _r(onorm[p][:, 128 * u:128 * (u + 1)]),
                                _r(wo_s[:, C * p + 512 * n2:C * p + 512 * (n2 + 1)]),
                                start=(p == 0), stop=(p == 1))
                        out_t = outpool.tile([128, 512], F32, tag="out")
                        nc.vector.tensor_copy(out_t[:], y_ps[:])
                        nc.sync.dma_start(
                            out=y_d.ap()[512 * j + 128 * u:512 * j + 128 * (u + 1),
                                         512 * n2:512 * (n2 + 1)],
                            in_=out_t[:])

            for j in range(NQ):
                emit_qkv_block(j)
                emit_attention_block(j)

    nc.compile()
    return nc


def _get_nc():
    if "nc" not in _CACHE:
        _CACHE["nc"] = _build()
    return _CACHE["nc"]


def _masks():
    if "ms" not in _CACHE:
        k = np.arange(128)[:, None]
        q = np.arange(512)[None, :]
        ms = np.zeros((4, 128, 1024), np.float32)
        for m in range(4):
            blk = (q >= k + 128 * m).astype(np.float32)
            ms[m, :, :512] = blk
            ms[m, :, 512:] = blk
        _CACHE["ms"] = ms.astype(ml_dtypes.bfloat16)
    return _CACHE["ms"]


def kernel(x, Wq, bq, Wk, bk, Wv, bv, Wo, bo):
    x = np.asarray(x, np.float32)
    Wq, bq = np.asarray(Wq, np.float32), np.asarray(bq, np.float32)
    Wk, bk = np.asarray(Wk, np.float32), np.asarray(bk, np.float32)
    Wv, bv = np.asarray(Wv, np.float32), np.asarray(bv, np.float32)
    Wo, bo = np.asarray(Wo, np.float32), np.asarray(bo, np.float32)

    nc = _get_nc()
    ms = _masks()

    ones = np.ones((128, 64), np.float32)

    def wv_host(w):
        out = np.zeros((C, HSV), np.float32)
        for h in range(4):
            out[:, 65 * h:65 * h + 64] = w[:, 64 * h:64 * (h + 1)]
        return out

    def bv_host(b):
        out = np.zeros((128, HSV), np.float32)
        for h in range(4):
            out[:, 65 * h:65 * h + 64] = b[64 * h:64 * (h + 1)][None, :]
            out[:, 65 * h + 64] = 1.0
        return out

    in_maps = []
    for core in range(N_CORES):
        b, g = divmod(core, 4)
        sl = slice(HS * g, HS * (g + 1))
        in_maps.append({
            "xt": np.ascontiguousarray(x[b].T),
            "wq": np.ascontiguousarray(Wq[:, sl]) * 0.125,
            "wk": np.ascontiguousarray(Wk[:, sl]),
            "wv": wv_host(Wv[:, sl]),
            "wo": np.ascontiguousarray(Wo[sl, :]),
            "bq": (bq[sl] * 0.125).reshape(2, 128).T.copy(),
            "bk": bk[sl].reshape(2, 128).T.copy(),
            "bv": bv_host(bv[sl]),
            "ms": ms,
            "on": ones,
        })

    res = run_bass_kernel_spmd(nc, in_maps, core_ids=list(range(N_CORES)),
                               **_CACHE.get("run_kwargs", {}))
    _CACHE["last_result"] = res

    y = np.zeros((B, T, C), np.float32)
    for core in range(N_CORES):
        b = core // 4
        y[b] += res.results[core]["y"]
    y += bo
    return y
